# revision 26
# baseline (speedup 1.0000x reference)
"""NT-Xent loss on 8 Trainium2 NeuronCores — triangular fp8 scheme.

Math: xn = row-normalized x; mat = exp(xn @ xn.T / 0.1) with zero diag;
numer_r = mat[r, (r+B) mod N]; denom_r = column sum r (= row sum r, mat
symmetric); loss = -mean(log(numer/denom)).

Work assignment (circulant triangle): core c owns row block c (1024 rows,
input rolled by -1024c so everything is SPMD-uniform) and computes only
column pairs j = 0..4 (local cols 0..5119), i.e. blocks (c, c+j mod 8).
Row block b then recovers its full denominator from
  - its own row sums over pairs 0..4      (cols b..b+4)
  - COLUMN sums of blocks (b-j, b), j=1..3, computed on cores b-1..b-3
    as the column sums of their pairs 1..3 (mat symmetry).
Pair 4 (c, c+4) is computed redundantly by both partner cores so no
colsum exchange is needed for it. The colsum partials ([128,1024] per
pair, partition-summed on the host) plus own rowsums are combined on the
host, which already does the final log/mean — no device collectives.

Precision: operands are fp8 e4m3 (xn * 16), matmul accumulates fp32 in
PSUM via DoubleRow perf mode (2 k-tiles per instruction, 2x PE rate),
exp runs on ACT straight from PSUM with scale 10/256 and accum_out row
sums. diag/positive-pair values are extracted from the bf16 exp tiles
with batched strided-window APs against a broadcast identity; the diag
subtraction cancels to bf16 noise and fp8 quantization noise averages
out across 8192 rows (< 1e-3 rel).

Per-core pipeline (engines in steady state):
  DMA   five x-tile group loads [128, 8x512] bf16, alternating queues.
  DVE   squares + row-reduce (bf16 2x) -> ss; 2-step Newton rsqrt
        (bit-trick seed, *16 folded) -> invn16; one broadcast multiply
        per group -> diag(invn16) tiles; colsum accumulation; extracts.
  PE    transpose fills (xn @ diag -> PSUM [128,2048]) and DoubleRow
        mains ([128, 1024/2048] per (m, pair-group)).
  ACT   Exp from PSUM with accum_out rowsums; half the transpose drains
        (fp8 cast); DVE takes the other half.
"""

import functools

import ml_dtypes
import numpy as np

N, D, B = 8192, 512, 4096
NCORES = 8
RPC = N // NCORES           # 1024 rows per core
PAIRS = 5                   # column pairs computed per core
TILES = 8 * PAIRS           # 40 row tiles of rolled x
ROWS_IN = TILES * 128       # 5120 input rows per core
MB = RPC // 128             # 8 row blocks of 128
SCALE = 16.0                # fp8 operand scale
EXPS = 10.0 / (SCALE * SCALE)  # activation scale: 1/temp / SCALE^2
PGROUPS = ((0,), (1, 2), (3, 4))


def _build():
    from contextlib import ExitStack

    import concourse.bacc as bacc
    import concourse.mybir as mybir
    import concourse.tile as tile

    F32 = mybir.dt.float32
    BF16 = mybir.dt.bfloat16
    FP8 = mybir.dt.float8e4
    I32 = mybir.dt.int32
    ALU = mybir.AluOpType
    ACTF = mybir.ActivationFunctionType
    AX = mybir.AxisListType
    DR = mybir.MatmulPerfMode.DoubleRow

    nc = bacc.Bacc("TRN2", target_bir_lowering=False, debug=False,
                   num_devices=NCORES)
    x_in = nc.dram_tensor("x", [ROWS_IN, D], BF16, kind="ExternalInput").ap()
    eye16_in = nc.dram_tensor("eye16", [128, 128], BF16, kind="ExternalInput").ap()
    numer_out = nc.dram_tensor("numer", [128, MB], F32, kind="ExternalOutput").ap()
    denom_out = nc.dram_tensor("denom", [128, MB], F32, kind="ExternalOutput").ap()
    cs12_out = nc.dram_tensor("cs12", [128, 2 * RPC], BF16,
                              kind="ExternalOutput").ap()
    cs3_out = nc.dram_tensor("cs3", [128, RPC], BF16,
                             kind="ExternalOutput").ap()

    with ExitStack() as ctx:
        tc = ctx.enter_context(tile.TileContext(nc))
        consts = ctx.enter_context(tc.tile_pool(name="consts", bufs=1))
        xldp = ctx.enter_context(tc.tile_pool(name="xld", bufs=1))
        stats = ctx.enter_context(tc.tile_pool(name="stats", bufs=1))
        xtp = ctx.enter_context(tc.tile_pool(name="xt", bufs=1))
        eop = ctx.enter_context(tc.tile_pool(name="eo", bufs=4))
        eobig = ctx.enter_context(tc.tile_pool(name="eobig", bufs=1))
        colp = ctx.enter_context(tc.tile_pool(name="col", bufs=1))
        junkp = ctx.enter_context(tc.tile_pool(name="junk", bufs=2))
        psm = ctx.enter_context(tc.tile_pool(name="psm", bufs=2, space="PSUM"))

        eye16 = consts.tile([128, 128], BF16, tag="eye16")
        nc.sync.dma_start(eye16[:], eye16_in)

        xld = [xldp.tile([128, 8 * D], BF16, tag=f"xld{g}", name=f"xld{g}")
               for g in range(PAIRS)]

        def xn(t):  # [128, 512] view of row tile t
            return xld[t // 8][:, (t % 8) * D:(t % 8 + 1) * D]

        ss = stats.tile([128, TILES], F32, tag="ss")
        invn16 = stats.tile([128, TILES], BF16, tag="invn16")
        iu = stats.tile([128, TILES], I32, tag="iu")
        iv = stats.tile([128, TILES], I32, tag="iv")
        nt_t = stats.tile([128, TILES], F32, tag="nt_t")
        dts = [stats.tile([128, 1024], BF16, tag=f"dts{g}", name=f"dts{g}")
               for g in range(PAIRS)]
        rs = stats.tile([128, MB * 3], F32, tag="rs")
        dxv = stats.tile([128, MB], F32, tag="dxv")
        nxv = stats.tile([128, MB], F32, tag="nxv")
        rowsum = stats.tile([128, MB], F32, tag="rowsum")
        dent = stats.tile([128, MB], F32, tag="dent")

        xt = [xtp.tile([128, 4 * 1024], FP8, tag=f"xt{j}", name=f"xt{j}")
              for j in range(PAIRS)]
        colacc12 = colp.tile([128, 2048], BF16, tag="cacc12")
        colacc3 = colp.tile([128, 1024], BF16, tag="cacc3")
        # exp tiles for pair groups 0 and (3,4) persist (padded for the
        # strided diag-window extraction); group (1,2) cycles a ring.
        eo0 = eobig.tile([128, 8 * 1024 + 1024], BF16, tag="eo0")
        eo34 = eobig.tile([128, 8 * 2048 + 1024], BF16, tag="eo34")

        # Input loads: one DMA per 8-tile group, alternating issue queues.
        # Row-interleaved layout: xld[g][p, s*512+d] = x[g*1024 + 8p + s, d],
        # so each partition's 8x512 span is one contiguous 8 KB DMA line.
        # This permutes "tile s" to the stride-8 row set {8p+s}; the same
        # permutation lands on both matmul operands (lhsT and rhs come from
        # the same transposed tiles), so diagonals stay diagonal on device
        # and only the host-side index mapping changes (r = 8p + s).
        for g in range(PAIRS):
            src = x_in[g * 1024:(g + 1) * 1024, :].rearrange(
                "(p s) d -> p (s d)", s=8)
            # half-loads on both HWDGE rings so transfers run in parallel
            nc.sync.dma_start(xld[g][0:64, :], src[0:64, :])
            nc.scalar.dma_start(xld[g][64:128, :], src[64:128, :])

        def norm_group(g, act_tiles=0):
            """Row sum-of-squares (split DVE/ACT), Newton rsqrt, diag tiles."""
            sl = slice(8 * g, 8 * g + 8)
            for q in range(8):
                t = 8 * g + q
                if q < act_tiles:
                    ja = junkp.tile([128, D], BF16, tag="na", name=f"na{t}")
                    nc.scalar.activation(ja[:], xn(t), ACTF.Square,
                                         accum_out=ss[:, t:t + 1])
                else:
                    jb = junkp.tile([128, D], BF16, tag="nj", name=f"nj{t}")
                    nc.vector.scalar_tensor_tensor(
                        jb[:], xn(t), 1.0, xn(t),
                        op0=ALU.mult, op1=ALU.mult, accum_out=ss[:, t:t + 1])
            nc.vector.tensor_scalar(iu[:, sl], ss[:, sl].bitcast(I32), 1, None,
                                    op0=ALU.arith_shift_right)
            nc.vector.tensor_scalar(iv[:, sl], iu[:, sl], -1, 0x5F3759DF,
                                    op0=ALU.mult, op1=ALU.add)
            y = iv[:, sl].bitcast(F32)
            for it in range(2):
                nc.vector.tensor_mul(nt_t[:, sl], y, y)
                nc.vector.tensor_mul(nt_t[:, sl], nt_t[:, sl], ss[:, sl])
                if it == 0:
                    nc.vector.tensor_scalar(nt_t[:, sl], nt_t[:, sl], -0.5, 1.5,
                                            op0=ALU.mult, op1=ALU.add)
                    nc.vector.tensor_mul(y, y, nt_t[:, sl])
                else:
                    # fold the fp8 operand scale into the last iteration
                    nc.vector.tensor_scalar(nt_t[:, sl], nt_t[:, sl],
                                            -0.5 * SCALE, 1.5 * SCALE,
                                            op0=ALU.mult, op1=ALU.add)
                    nc.vector.tensor_mul(invn16[:, sl], y, nt_t[:, sl])
            # dts[g][:, q*128+c] = (p==c) * invn16[p, 8g+q]
            nc.vector.tensor_tensor(
                dts[g][:].rearrange("p (s c) -> p s c", s=8),
                eye16[:, None, :].to_broadcast([128, 8, 128]),
                invn16[:, sl, None].to_broadcast([128, 8, 128]),
                op=ALU.mult)

        def transpose_pair(j):
            """xt[j][:, k*1024 + tt*128 + r] = xn(8j+tt)[r, k*128+d]*invn16."""
            for kk in range(2):
                ps = psm.tile([128, 2048], F32, tag="ps", name=f"tf{j}_{kk}")
                for tt in range(8):
                    t = 8 * j + tt
                    for ks in range(2):
                        k = 2 * kk + ks
                        nc.tensor.matmul(
                            ps[:, ks * 1024 + tt * 128:ks * 1024 + (tt + 1) * 128],
                            lhsT=xn(t)[:, k * 128:(k + 1) * 128],
                            rhs=dts[j][:, tt * 128:(tt + 1) * 128],
                            start=True, stop=True)
                dst = xt[j][:, kk * 2048:(kk + 1) * 2048]
                if kk == 0:
                    nc.scalar.copy(dst, ps[:])
                else:
                    nc.vector.tensor_copy(dst, ps[:])

        def dr_ap(j, kk, lo, w):
            """[128, 2, w] DoubleRow AP over xt[j], k-slices {2kk, 2kk+1}."""
            return xt[j][:, kk * 2048:(kk + 1) * 2048].rearrange(
                "p (two c) -> p two c", two=2)[:, :, lo:lo + w]

        def main_fill(m, pgi):
            pg = PGROUPS[pgi]
            w = 1024 * len(pg)
            ps = psm.tile([128, 2048], F32, tag="ps", name=f"mf{m}_{pgi}")
            for ji, j in enumerate(pg):
                for h in range(2):
                    for kk in range(2):
                        nc.tensor.matmul(
                            ps[:, ji * 1024 + h * 512:ji * 1024 + (h + 1) * 512],
                            lhsT=dr_ap(0, kk, m * 128, 128),
                            rhs=dr_ap(j, kk, h * 512, 512),
                            start=(kk == 0), stop=(kk == 1),
                            perf_mode=DR)
            if pgi == 0:
                eo = eo0[:, m * 1024:(m + 1) * 1024]
            elif pgi == 2:
                eo = eo34[:, m * 2048:(m + 1) * 2048]
            else:
                eo = eop.tile([128, 2048], BF16, tag="eo", name=f"eo{m}")[:, :w]
            nc.scalar.activation(eo, ps[:, :w], ACTF.Exp, scale=EXPS,
                                 accum_out=rs[:, m * 3 + pgi:m * 3 + pgi + 1])
            if pgi == 1:
                if m == 0:
                    nc.vector.tensor_copy(colacc12[:], eo)
                else:
                    nc.vector.tensor_add(colacc12[:], colacc12[:], eo)
            elif pgi == 2:
                src = eo34[:, m * 2048:m * 2048 + 1024]
                if m == 0:
                    nc.vector.tensor_copy(colacc3[:], src)
                else:
                    nc.vector.tensor_add(colacc3[:], colacc3[:], src)

        # ---- batched diag/positive extraction from the bf16 exp tiles ----
        # window m of the padded eo tile has its diag block at a constant
        # offset; junk = window * eye (partition-broadcast), reduce -> [128,8]
        def extract(eo_all, stride, off, tgt):
            win = eo_all[:].rearrange("p (m c) -> p m c", c=stride)[
                :, :, off:off + 128]
            jx = junkp.tile([128, 8 * 128], BF16, tag="xj", name="xj")
            nc.vector.tensor_tensor(
                jx[:].rearrange("p (m c) -> p m c", m=8), win,
                eye16[:, None, :].to_broadcast([128, 8, 128]), op=ALU.mult)
            nc.vector.tensor_reduce(tgt[:],
                                    jx[:].rearrange("p (m c) -> p m c", m=8),
                                    axis=AX.X, op=ALU.add)

        # ---- emission schedule (pipelined) ----
        norm_group(0, act_tiles=4)
        transpose_pair(0)
        norm_group(1, act_tiles=4)
        for m in range(2):
            main_fill(m, 0)
        transpose_pair(1)
        norm_group(2)
        for m in range(2, 5):
            main_fill(m, 0)
        transpose_pair(2)
        norm_group(3)
        for m in range(5, 8):
            main_fill(m, 0)
        extract(eo0, 1152, 0, dxv)          # diag: col m*128 in pair-0 block
        for m in range(2):
            main_fill(m, 1)
        transpose_pair(3)
        norm_group(4)
        for m in range(2, 5):
            main_fill(m, 1)
        transpose_pair(4)
        for m in range(5, 8):
            main_fill(m, 1)
        nc.sync.dma_start(cs12_out, colacc12[:])
        for m in range(8):
            main_fill(m, 2)

        extract(eo34, 2176, 1024, nxv)      # positive: col m*128 in pair-4

        # ---- finalize ----
        nc.vector.tensor_reduce(rowsum[:], rs[:].rearrange("p (m q) -> p m q", m=MB),
                                axis=AX.X, op=ALU.add)
        nc.vector.tensor_sub(dent[:], rowsum[:], dxv[:])
        nc.sync.dma_start(numer_out, nxv[:])
        nc.sync.dma_start(denom_out, dent[:])
        nc.scalar.dma_start(cs3_out, colacc3[:])

    nc.finalize()
    return nc


@functools.lru_cache(maxsize=1)
def _get_nc():
    return _build()


def _in_maps(x):
    x = np.asarray(x)
    assert x.shape == (N, D)
    xb = np.asarray(x, dtype=np.float32).astype(ml_dtypes.bfloat16)
    eye16 = np.eye(128, dtype=ml_dtypes.bfloat16)
    return [
        {"x": np.ascontiguousarray(np.roll(xb, -c * RPC, axis=0)[:ROWS_IN]),
         "eye16": eye16}
        for c in range(NCORES)
    ]


def _run(x, **run_kwargs):
    from concourse.bass_utils import run_bass_kernel_spmd

    nc = _get_nc()
    return run_bass_kernel_spmd(nc, _in_maps(x), list(range(NCORES)),
                                **run_kwargs)


def _loss_from_results(results):
    # Device row/col position q = s*128 + i maps to local row r = 8i + s
    # (row-interleaved load layout). numer/denom [128(p=i), 8(m=s)]
    # flatten as r = 8p + m, i.e. a plain reshape. Column positions use
    # PERM[q] = local row index.
    r = np.arange(RPC)
    perm = (r % 8) * 128 + r // 8  # device position of local row r
    num = np.concatenate(
        [results[c]["numer"].reshape(-1).astype(np.float64)
         for c in range(NCORES)])
    den_own = [results[c]["denom"].reshape(-1).astype(np.float64)
               for c in range(NCORES)]
    # colsum partials: cs12 holds pairs 1,2; cs3 pair 3. Partition-sum on
    # host; pair j of core c belongs to rows of core (c+j) mod 8.
    cs = [np.concatenate(
        [np.asarray(results[c]["cs12"], dtype=np.float64),
         np.asarray(results[c]["cs3"], dtype=np.float64)], axis=1)
        for c in range(NCORES)]
    den = np.concatenate([
        den_own[b] + sum(
            cs[(b - j) % NCORES][:, (j - 1) * RPC:j * RPC].sum(axis=0)[perm]
            for j in range(1, 4))
        for b in range(NCORES)])
    loss = -np.sum(np.log(num / den)) / N
    return np.float32(loss)


def kernel(x):
    res = _run(x)
    return _loss_from_results(res.results)
